# revision 1
# baseline (speedup 1.0000x reference)
"""CP-decomposed 4D linear layer on 8 Trainium2 NeuronCores.

out[b, cls] = sum_r lam[r] * U4[cls,r] * sum_c U3[c,r] * sum_w U2[w,r] * sum_h U1[h,r] * x[b,c,w,h]

Strategy (data-parallel over batch, 16 b per core):
  - host precomputes G[r, w*32+h] = U2[w,r]*U1[h,r]  (64 x 1024)
    and A[r, cls] = lam[r]*U4[cls,r]                  (64 x 1000)
  - per b: PE contracts c (K=512 in 4 chunks of 128):
        t[r, f] = sum_c U3[c,r] * x[b,c,f]   -> PSUM [64, 1024]  (float32r matmul)
  - DVE fused multiply-reduce: z[r, b] = sum_f t[r,f]*G[r,f]     -> zbuf column
  - final PE matmul: out[b, cls] = sum_r zbuf[r,b] * A[r,cls]    (fp32)
"""

import numpy as np

import concourse.bass as bass
import concourse.bacc as bacc
import concourse.mybir as mybir
import concourse.tile as tile
from concourse.bass_utils import run_bass_kernel_spmd

B, C, W, H, CLS, R = 128, 512, 32, 32, 1000, 64
WH = W * H          # 1024
N_CORES = 8
B_LOC = B // N_CORES  # 16
KC = C // 128         # 4 contraction chunks
MM_DT = mybir.dt.float32r   # step-A matmul dtype (full PE rate, reduced precision)
F32 = mybir.dt.float32

_NC_CACHE = {}


def _pe_dep_nop(nc, in_aps, out_aps=()):
    """NoOp on the PE queue declaring deps, so Tile attaches semaphore waits
    here instead of on a Matmult (whose LW struct only takes one wait)."""
    inst = mybir.InstNoOp(
        name=nc.get_next_instruction_name(), text_hint="dep", bass_nofuse=True
    )
    inst.engine = mybir.EngineType.PE
    inst.ins = [nc.tensor.lower_ap(ap) for ap in in_aps]
    inst.outs = [nc.tensor.lower_ap(ap) for ap in out_aps]
    nc.add_instruction(inst)


def _build(mm_dt=MM_DT, reps=1, xbufs=3, dma_only=False):
    nc = bacc.Bacc()
    x = nc.declare_dram_parameter("x", [B_LOC, C, WH], mm_dt, isOutput=False)
    u3 = nc.declare_dram_parameter("u3", [C, R], mm_dt, isOutput=False)
    g = nc.declare_dram_parameter("g", [R, WH], F32, isOutput=False)
    a = nc.declare_dram_parameter("a", [R, CLS], F32, isOutput=False)
    out = nc.declare_dram_parameter("out", [B_LOC, CLS], F32, isOutput=True)

    with tile.TileContext(nc) as tc:
        with (
            tc.tile_pool(name="const", bufs=1) as cpool,
            tc.tile_pool(name="xp", bufs=xbufs) as xpool,
            tc.tile_pool(name="tmp", bufs=2) as tpool,
            tc.tile_pool(name="ps", bufs=2, space="PSUM") as pspool,
            tc.tile_pool(name="psd", bufs=1, space="PSUM") as psdpool,
        ):
            # constants (replicated factors)
            u3s = cpool.tile([128, KC, R], mm_dt)
            nc.sync.dma_start(u3s[:], u3.rearrange("(k p) r -> p k r", p=128))
            gs = cpool.tile([R, WH], F32)
            nc.sync.dma_start(gs[:], g[:])
            asb = cpool.tile([R, CLS], F32)
            nc.sync.dma_start(asb[:], a[:])
            zbuf = cpool.tile([R, B_LOC], F32)

            # x[b] viewed as [p(128), k(4), f(1024)] with c = k*128 + p
            xr = x.rearrange("b (k p) f -> b p k f", p=128)

            for rep in range(reps):
              for b in range(B_LOC):
                xb = xpool.tile([128, KC, WH], mm_dt, tag="xb")
                nc.sync.dma_start(xb[:], xr[b])

                if dma_only:
                    continue
                tps = pspool.tile([64, WH], F32, tag="tps")
                for c in range(KC):
                    for n in range(2):
                        sl = bass.ts(n, 512)
                        nc.tensor.matmul(
                            tps[:, sl],
                            u3s[:, c, :],
                            xb[:, c, sl],
                            start=(c == 0),
                            stop=(c == KC - 1),
                        )

                tmp = tpool.tile([R, WH], F32, tag="ttr")
                nc.vector.tensor_tensor(tmp[:], tps[:], gs[:], mybir.AluOpType.mult)
                nc.vector.tensor_reduce(
                    zbuf[:, b : b + 1],
                    tmp[:],
                    mybir.AxisListType.X,
                    mybir.AluOpType.add,
                )

            # step D: out[b, cls] = sum_r zbuf[r, b] * A[r, cls]
            od0 = psdpool.tile([B_LOC, 512], F32, tag="od0")
            od1 = psdpool.tile([B_LOC, 512], F32, tag="od1")
            nc.tensor.matmul(od0[:], zbuf[:], asb[:, 0:512], start=True, stop=True)
            nc.tensor.matmul(
                od1[:, 0 : CLS - 512], zbuf[:], asb[:, 512:CLS], start=True, stop=True
            )
            osb = cpool.tile([B_LOC, CLS], F32)
            nc.vector.tensor_copy(osb[:, 0:512], od0[:])
            nc.vector.tensor_copy(osb[:, 512:CLS], od1[:, 0 : CLS - 512])
            nc.sync.dma_start(out[:], osb[:])

    nc.compile()
    return nc


def _get_nc(mm_dt=MM_DT):
    key = str(mm_dt)
    if key not in _NC_CACHE:
        _NC_CACHE[key] = _build(mm_dt)
    return _NC_CACHE[key]


def _prep_inputs(x, U1, U2, U3, U4, lam):
    x = np.ascontiguousarray(np.asarray(x, dtype=np.float32)).reshape(B, C, WH)
    U1 = np.asarray(U1, dtype=np.float32)
    U2 = np.asarray(U2, dtype=np.float32)
    U3 = np.ascontiguousarray(np.asarray(U3, dtype=np.float32))
    U4 = np.asarray(U4, dtype=np.float32)
    lam = np.asarray(lam, dtype=np.float32)

    # G[r, w*32+h] = U2[w,r] * U1[h,r]
    G = np.ascontiguousarray(
        (U2.T[:, :, None] * U1.T[:, None, :]).reshape(R, WH).astype(np.float32)
    )
    # A[r, cls] = lam[r] * U4[cls, r]
    A = np.ascontiguousarray((U4 * lam[None, :]).T.astype(np.float32))

    in_maps = [
        {
            "x": np.ascontiguousarray(x[i * B_LOC : (i + 1) * B_LOC]),
            "u3": U3,
            "g": G,
            "a": A,
        }
        for i in range(N_CORES)
    ]
    return in_maps


def kernel(x, U1, U2, U3, U4, lam):
    in_maps = _prep_inputs(x, U1, U2, U3, U4, lam)
    nc = _get_nc()
    res = run_bass_kernel_spmd(nc, in_maps, list(range(N_CORES)))
    return np.concatenate([res.results[i]["out"] for i in range(N_CORES)], axis=0)


def kernel_timed(x, U1, U2, U3, U4, lam, mm_dt=MM_DT):
    """Run with NTFF tracing; returns (output, exec_time_ns)."""
    in_maps = _prep_inputs(x, U1, U2, U3, U4, lam)
    nc = _get_nc(mm_dt)
    res = run_bass_kernel_spmd(nc, in_maps, list(range(N_CORES)), trace=True)
    out = np.concatenate([res.results[i]["out"] for i in range(N_CORES)], axis=0)
    return out, res.exec_time_ns



# revision 2
# speedup vs baseline: 776.7134x; 776.7134x over previous
"""CP-decomposed 4D linear layer on 8 Trainium2 NeuronCores.

out[b, cls] = sum_r lam[r] * U4[cls,r] * sum_c U3[c,r] * sum_w U2[w,r] * sum_h U1[h,r] * x[b,c,w,h]

Strategy (data-parallel over batch, 16 b per core):
  - host precomputes G[r, w*32+h] = U2[w,r]*U1[h,r]  (64 x 1024)
    and A[r, cls] = lam[r]*U4[cls,r]                  (64 x 1000)
    and u3p[p, k, r] = U3[4p+k, r]                    (128 x 4 x 64)
  - x is streamed so partition p holds the 4 contiguous channel rows
    {4p..4p+3}: one 16 KB contiguous HBM read per partition per batch.
  - per b: PE contracts c (K=512 in 4 chunks of 128):
        t[r, f] = sum_c U3[c,r] * x[b,c,f]   -> PSUM [64, 1024]  (float32r matmul)
  - DVE multiply + reduce: z[r, b] = sum_f t[r,f]*G[r,f]
  - final PE matmul: out[b, cls] = sum_r z[r,b] * A[r,cls]

The kernel is DMA-bound: 32 MB of x per core at ~358 GB/s/core HBM
bandwidth gives an ~89 us floor; measured device time is ~90 us with
PE (~40%) and DVE (~35%) hidden underneath the x stream.
"""

import numpy as np

import concourse.bass as bass
import concourse.bacc as bacc
import concourse.mybir as mybir
import concourse.tile as tile
from concourse.bass_utils import run_bass_kernel_spmd

B, C, W, H, CLS, R = 128, 512, 32, 32, 1000, 64
WH = W * H            # 1024
N_CORES = 8
B_LOC = B // N_CORES  # 16
KC = C // 128         # 4 contraction chunks
MM_DT = mybir.dt.float32r   # step-A matmul dtype (full PE rate, reduced precision)
F32 = mybir.dt.float32

_NC_CACHE = {}


def _build(reps=1, xbufs=3):
    """Build the kernel; `reps` repeats the FULL computation (all DMA +
    compute + output store) that many times inside one NEFF — used by
    test.py to amortize per-dispatch overhead when timing."""
    nc = bacc.Bacc()
    x = nc.declare_dram_parameter("x", [B_LOC, C, WH], MM_DT, isOutput=False)
    u3p = nc.declare_dram_parameter("u3p", [128, KC, R], MM_DT, isOutput=False)
    g = nc.declare_dram_parameter("g", [R, WH], F32, isOutput=False)
    a = nc.declare_dram_parameter("a", [R, CLS], F32, isOutput=False)
    out = nc.declare_dram_parameter("out", [B_LOC, CLS], F32, isOutput=True)

    # partition p holds channel rows {4p+k}: 16 KB contiguous per partition
    xr = x.rearrange("b (p k) f -> b p k f", p=128)

    with tile.TileContext(nc) as tc:
        with (
            tc.tile_pool(name="const", bufs=1) as cpool,
            tc.tile_pool(name="xp", bufs=xbufs) as xpool,
            tc.tile_pool(name="tmp", bufs=2) as tpool,
            tc.tile_pool(name="ps", bufs=2, space="PSUM") as pspool,
            tc.tile_pool(name="psd", bufs=1, space="PSUM") as psdpool,
        ):
            # constants (replicated factor matrices)
            u3s = cpool.tile([128, KC, R], MM_DT)
            nc.sync.dma_start(u3s[:], u3p[:])
            gs = cpool.tile([R, WH], F32)
            nc.sync.dma_start(gs[:], g[:])
            asb = cpool.tile([R, CLS], F32)
            nc.sync.dma_start(asb[:], a[:])
            zbuf = cpool.tile([R, B_LOC], F32)

            for rep in range(reps):
                for b in range(B_LOC):
                    xb = xpool.tile([128, KC, WH], MM_DT, tag="xb")
                    nc.sync.dma_start(xb[:], xr[b])

                    tps = pspool.tile([R, WH], F32, tag="tps")
                    for k in range(KC):
                        for n in range(2):
                            sl = bass.ts(n, 512)
                            nc.tensor.matmul(
                                tps[:, sl],
                                u3s[:, k, :],
                                xb[:, k, sl],
                                start=(k == 0),
                                stop=(k == KC - 1),
                            )

                    tmp = tpool.tile([R, WH], F32, tag="ttr")
                    nc.vector.tensor_tensor(
                        tmp[:], tps[:], gs[:], mybir.AluOpType.mult
                    )
                    nc.vector.tensor_reduce(
                        zbuf[:, b : b + 1],
                        tmp[:],
                        mybir.AxisListType.X,
                        mybir.AluOpType.add,
                    )

                # out[b, cls] = sum_r zbuf[r, b] * A[r, cls]
                od0 = psdpool.tile([B_LOC, 512], F32, tag="od0")
                od1 = psdpool.tile([B_LOC, 512], F32, tag="od1")
                nc.tensor.matmul(od0[:], zbuf[:], asb[:, 0:512], start=True, stop=True)
                nc.tensor.matmul(
                    od1[:, 0 : CLS - 512], zbuf[:], asb[:, 512:CLS],
                    start=True, stop=True,
                )
                osb = cpool.tile([B_LOC, CLS], F32)
                nc.vector.tensor_copy(osb[:, 0:512], od0[:])
                nc.vector.tensor_copy(osb[:, 512:CLS], od1[:, 0 : CLS - 512])
                nc.sync.dma_start(out[:], osb[:])

    nc.compile()
    return nc


def _get_nc(reps=1):
    if reps not in _NC_CACHE:
        _NC_CACHE[reps] = _build(reps=reps)
    return _NC_CACHE[reps]


def _prep_inputs(x, U1, U2, U3, U4, lam):
    x = np.ascontiguousarray(np.asarray(x, dtype=np.float32)).reshape(B, C, WH)
    U1 = np.asarray(U1, dtype=np.float32)
    U2 = np.asarray(U2, dtype=np.float32)
    U3 = np.asarray(U3, dtype=np.float32)
    U4 = np.asarray(U4, dtype=np.float32)
    lam = np.asarray(lam, dtype=np.float32)

    # G[r, w*32+h] = U2[w,r] * U1[h,r]
    G = np.ascontiguousarray(
        (U2.T[:, :, None] * U1.T[:, None, :]).reshape(R, WH).astype(np.float32)
    )
    # A[r, cls] = lam[r] * U4[cls, r]
    A = np.ascontiguousarray((U4 * lam[None, :]).T.astype(np.float32))
    # u3p[p, k, r] = U3[4p+k, r]
    u3p = np.ascontiguousarray(U3.reshape(128, KC, R))

    in_maps = [
        {
            "x": np.ascontiguousarray(x[i * B_LOC : (i + 1) * B_LOC]),
            "u3p": u3p,
            "g": G,
            "a": A,
        }
        for i in range(N_CORES)
    ]
    return in_maps


def _unshard(outs):
    """outs: list of per-core [B_LOC, CLS] arrays -> full [B, CLS]."""
    return np.concatenate(list(outs), axis=0)


def kernel(x, U1, U2, U3, U4, lam):
    in_maps = _prep_inputs(x, U1, U2, U3, U4, lam)
    nc = _get_nc()
    res = run_bass_kernel_spmd(nc, in_maps, list(range(N_CORES)))
    return _unshard(res.results[i]["out"] for i in range(N_CORES))


# revision 4
# speedup vs baseline: 809.9347x; 1.0428x over previous
"""CP-decomposed 4D linear layer on 8 Trainium2 NeuronCores.

out[b, cls] = sum_r lam[r] * U4[cls,r] * sum_c U3[c,r] * sum_w U2[w,r] * sum_h U1[h,r] * x[b,c,w,h]

Strategy (data-parallel over batch, 16 b per core):
  - host precomputes G[r, w*32+h] = U2[w,r]*U1[h,r]  (64 x 1024)
    and A[r, cls] = lam[r]*U4[cls,r]                  (64 x 1000)
    and u3p[p, k, r] = U3[4p+k, r]                    (128 x 4 x 64)
  - x is streamed so partition p holds the 4 contiguous channel rows
    {4p..4p+3}: one 16 KB contiguous HBM read per partition per batch.
  - per b: PE contracts c (K=512 in 4 chunks of 128):
        t[r, f] = sum_c U3[c,r] * x[b,c,f]   -> PSUM [64, 1024]  (float32r matmul)
  - DVE multiply + reduce: z[r, b] = sum_f t[r,f]*G[r,f]
  - final PE matmul: out[b, cls] = sum_r z[r,b] * A[r,cls]

The kernel is DMA-bound: 32 MB of x per core at ~358 GB/s/core HBM
bandwidth gives an ~89 us floor; measured device time is ~90 us with
PE (~40%) and DVE (~35%) hidden underneath the x stream.
"""

import numpy as np

import concourse.bass as bass
import concourse.bacc as bacc
import concourse.mybir as mybir
import concourse.tile as tile
from concourse.bass_utils import run_bass_kernel_spmd

B, C, W, H, CLS, R = 128, 512, 32, 32, 1000, 64
WH = W * H            # 1024
N_CORES = 8
B_LOC = B // N_CORES  # 16
KC = C // 128         # 4 contraction chunks
MM_DT = mybir.dt.float32r   # step-A matmul dtype (full PE rate, reduced precision)
F32 = mybir.dt.float32

_NC_CACHE = {}


def _build(reps=1, xbufs=3):
    """Build the kernel; `reps` repeats the FULL computation (all DMA +
    compute + output store) that many times inside one NEFF — used by
    test.py to amortize per-dispatch overhead when timing."""
    nc = bacc.Bacc()
    x = nc.declare_dram_parameter("x", [B_LOC, C, WH], MM_DT, isOutput=False)
    u3p = nc.declare_dram_parameter("u3p", [128, KC, R], MM_DT, isOutput=False)
    g = nc.declare_dram_parameter("g", [R, WH], F32, isOutput=False)
    a = nc.declare_dram_parameter("a", [R, CLS], F32, isOutput=False)
    out = nc.declare_dram_parameter("out", [B_LOC, CLS], F32, isOutput=True)

    # partition p holds channel rows {4p+k}: 16 KB contiguous per partition;
    # 2 batches per transfer (4 MB) to amortize DMA fixed costs
    xr = x.rearrange("(j b2) (p k) f -> j p b2 k f", b2=2, p=128)

    with tile.TileContext(nc) as tc:
        with (
            tc.tile_pool(name="const", bufs=1) as cpool,
            tc.tile_pool(name="xp", bufs=xbufs) as xpool,
            tc.tile_pool(name="tmp", bufs=2) as tpool,
            tc.tile_pool(name="ps", bufs=2, space="PSUM") as pspool,
            tc.tile_pool(name="psd", bufs=1, space="PSUM") as psdpool,
        ):
            # constants (replicated factor matrices)
            u3s = cpool.tile([128, KC, R], MM_DT)
            nc.sync.dma_start(u3s[:], u3p[:])
            gs = cpool.tile([R, WH], F32)
            nc.sync.dma_start(gs[:], g[:])
            asb = cpool.tile([R, CLS], F32)
            nc.sync.dma_start(asb[:], a[:])
            zbuf = cpool.tile([R, B_LOC], F32)

            for rep in range(reps):
                for j in range(B_LOC // 2):
                    xb = xpool.tile([128, 2, KC, WH], MM_DT, tag="xb")
                    nc.sync.dma_start(xb[:], xr[j])

                    for b2 in range(2):
                        b = 2 * j + b2
                        tps = pspool.tile([R, WH], F32, tag="tps")
                        for k in range(KC):
                            for n in range(2):
                                sl = bass.ts(n, 512)
                                nc.tensor.matmul(
                                    tps[:, sl],
                                    u3s[:, k, :],
                                    xb[:, b2, k, sl],
                                    start=(k == 0),
                                    stop=(k == KC - 1),
                                )

                        tmp = tpool.tile([R, WH], F32, tag="ttr")
                        nc.vector.tensor_tensor(
                            tmp[:], tps[:], gs[:], mybir.AluOpType.mult
                        )
                        nc.vector.tensor_reduce(
                            zbuf[:, b : b + 1],
                            tmp[:],
                            mybir.AxisListType.X,
                            mybir.AluOpType.add,
                        )

                # out[b, cls] = sum_r zbuf[r, b] * A[r, cls]
                od0 = psdpool.tile([B_LOC, 512], F32, tag="od0")
                od1 = psdpool.tile([B_LOC, 512], F32, tag="od1")
                nc.tensor.matmul(od0[:], zbuf[:], asb[:, 0:512], start=True, stop=True)
                nc.tensor.matmul(
                    od1[:, 0 : CLS - 512], zbuf[:], asb[:, 512:CLS],
                    start=True, stop=True,
                )
                osb = cpool.tile([B_LOC, CLS], F32)
                nc.vector.tensor_copy(osb[:, 0:512], od0[:])
                nc.vector.tensor_copy(osb[:, 512:CLS], od1[:, 0 : CLS - 512])
                nc.sync.dma_start(out[:], osb[:])

    nc.compile()
    return nc


def _get_nc(reps=1):
    if reps not in _NC_CACHE:
        _NC_CACHE[reps] = _build(reps=reps)
    return _NC_CACHE[reps]


def _prep_inputs(x, U1, U2, U3, U4, lam):
    x = np.ascontiguousarray(np.asarray(x, dtype=np.float32)).reshape(B, C, WH)
    U1 = np.asarray(U1, dtype=np.float32)
    U2 = np.asarray(U2, dtype=np.float32)
    U3 = np.asarray(U3, dtype=np.float32)
    U4 = np.asarray(U4, dtype=np.float32)
    lam = np.asarray(lam, dtype=np.float32)

    # G[r, w*32+h] = U2[w,r] * U1[h,r]
    G = np.ascontiguousarray(
        (U2.T[:, :, None] * U1.T[:, None, :]).reshape(R, WH).astype(np.float32)
    )
    # A[r, cls] = lam[r] * U4[cls, r]
    A = np.ascontiguousarray((U4 * lam[None, :]).T.astype(np.float32))
    # u3p[p, k, r] = U3[4p+k, r]
    u3p = np.ascontiguousarray(U3.reshape(128, KC, R))

    in_maps = [
        {
            "x": np.ascontiguousarray(x[i * B_LOC : (i + 1) * B_LOC]),
            "u3p": u3p,
            "g": G,
            "a": A,
        }
        for i in range(N_CORES)
    ]
    return in_maps


def _unshard(outs):
    """outs: list of per-core [B_LOC, CLS] arrays -> full [B, CLS]."""
    return np.concatenate(list(outs), axis=0)


def kernel(x, U1, U2, U3, U4, lam):
    in_maps = _prep_inputs(x, U1, U2, U3, U4, lam)
    nc = _get_nc()
    res = run_bass_kernel_spmd(nc, in_maps, list(range(N_CORES)))
    return _unshard(res.results[i]["out"] for i in range(N_CORES))
